# revision 1
# baseline (speedup 1.0000x reference)
"""GNN mean-aggregation (PyG MessagePassing, aggr='mean') on 8 TRN2 NeuronCores.

Sharding strategy (host): edges are partitioned by destination across the 8
cores (core c owns dst in [12500c, 12500(c+1))), and within each core further
partitioned into 98 sub-shards by 128-node destination block. Edges keep
their original relative order inside each sub-shard; sub-shards are padded to
a uniform chunk count so one compiled program serves every round/direction.

Device program "aggregate" (per core, per round):
  - indirect-DMA gather of x[src] rows (128 rows / instruction)
  - one-hot build for dst-lo via DVE is_equal against an iota tile
  - PE matmul accumulates one-hot^T @ msg into a PSUM-resident accumulator
    [128 lo x (98 blocks x 16 dims)]
  - counts come from running the same program with x = ones
Device program "divide": out = sums * reciprocal(max(cnt, 1)).
Host only reassembles the 8 per-core output slices into the full h between
rounds (allgather equivalent).
"""
import sys
sys.path.insert(0, '/opt/trn_rl_repo')
import numpy as np

import concourse.bass as bass
import concourse.tile as tile
from concourse import bacc, mybir
from concourse.bass_utils import run_bass_kernel_spmd

N_NODES = 100000
DIM = 16
N_EDGES = 3200000
N_CORES = 8
NLOC = N_NODES // N_CORES        # 12500 nodes per core
NBLK = (NLOC + 127) // 128       # 98 blocks per core
PAD_LO = 128                     # one-hot sentinel (never matches iota 0..127)

_PROGRAMS = {}


def _shard(edge_index):
    """Partition edges by (core, dst-block); pad sub-shards to uniform U."""
    src = np.asarray(edge_index[0], dtype=np.int64)
    dst = np.asarray(edge_index[1], dtype=np.int64)
    core = dst // NLOC
    loc = dst - core * NLOC
    blk = loc // 128
    lo = loc % 128
    key = core * NBLK + blk
    order = np.argsort(key, kind='stable')
    ks, ss, ls = key[order], src[order], lo[order]
    counts = np.bincount(ks, minlength=N_CORES * NBLK).reshape(N_CORES, NBLK)
    U = int((counts.max() + 127) // 128)
    gsrc = np.zeros((N_CORES, NBLK * U * 128), np.int32)
    glo = np.full((N_CORES, NBLK * U * 128), PAD_LO, np.float32)
    starts = np.zeros(N_CORES * NBLK + 1, np.int64)
    np.cumsum(counts.ravel(), out=starts[1:])
    for c in range(N_CORES):
        for b in range(NBLK):
            k = c * NBLK + b
            n = counts[c, b]
            s0 = starts[k]
            base = (b * U) * 128
            gsrc[c, base:base + n] = ss[s0:s0 + n]
            glo[c, base:base + n] = ls[s0:s0 + n]
    return gsrc, glo, U


def _build_aggregate(U):
    NS = NBLK * U                      # chunk slots per core
    nc = bacc.Bacc("TRN2", target_bir_lowering=False, debug=False,
                   num_devices=N_CORES)
    h_in = nc.dram_tensor("h", [N_NODES, DIM], mybir.dt.float32,
                          kind="ExternalInput")
    gsrc = nc.dram_tensor("gsrc", [128, NS], mybir.dt.int32,
                          kind="ExternalInput")   # slot-major, wrapped to 128 partitions
    iotaf = nc.dram_tensor("iotaf", [128, 128], mybir.dt.float32,
                           kind="ExternalInput")
    glo = nc.dram_tensor("glo", [128, NS], mybir.dt.float32,
                         kind="ExternalInput")
    sums = nc.dram_tensor("sums", [128, NBLK * DIM], mybir.dt.float32,
                          kind="ExternalOutput")
    with tile.TileContext(nc) as tc:
        with (
            tc.tile_pool(name="const", bufs=1) as constp,
            tc.tile_pool(name="idx", bufs=1) as idxp,
            tc.tile_pool(name="msg", bufs=24) as msgp,
            tc.tile_pool(name="oh", bufs=24) as ohp,
            tc.tile_pool(name="accs", bufs=1) as accp,
            tc.tile_pool(name="psum", bufs=1, space="PSUM") as psump,
        ):
            iota = constp.tile([128, 128], mybir.dt.float32)
            nc.sync.dma_start(out=iota[:], in_=iotaf.ap()[:])
            idx_t = idxp.tile([128, NS], mybir.dt.int32)
            nc.sync.dma_start(out=idx_t[:], in_=gsrc.ap()[:])
            lo_t = idxp.tile([128, NS], mybir.dt.float32)
            nc.sync.dma_start(out=lo_t[:], in_=glo.ap()[:])
            acc = psump.tile([128, NBLK * DIM], mybir.dt.float32, space="PSUM")
            for b in range(NBLK):
                for u in range(U):
                    s = b * U + u
                    msg = msgp.tile([128, DIM], mybir.dt.float32, tag="msg")
                    nc.gpsimd.indirect_dma_start(
                        out=msg[:], out_offset=None, in_=h_in.ap()[:],
                        in_offset=bass.IndirectOffsetOnAxis(
                            ap=idx_t[:, s:s + 1], axis=0))
                    oh = ohp.tile([128, 128], mybir.dt.float32, tag="oh")
                    nc.vector.tensor_tensor(
                        out=oh[:], in0=lo_t[:, s:s + 1].to_broadcast([128, 128]),
                        in1=iota[:], op=mybir.AluOpType.is_equal)
                    nc.tensor.matmul(
                        out=acc[:, b * DIM:(b + 1) * DIM], lhsT=oh[:], rhs=msg[:],
                        start=(u == 0), stop=(u == U - 1))
            accs = accp.tile([128, NBLK * DIM], mybir.dt.float32)
            nc.vector.tensor_copy(out=accs[:], in_=acc[:])
            nc.sync.dma_start(out=sums.ap()[:], in_=accs[:])
    nc.compile()
    return nc


def _build_divide():
    nc = bacc.Bacc("TRN2", target_bir_lowering=False, debug=False,
                   num_devices=N_CORES)
    s_in = nc.dram_tensor("s", [128, NBLK * DIM], mybir.dt.float32, kind="ExternalInput")
    c_in = nc.dram_tensor("c", [128, NBLK * DIM], mybir.dt.float32, kind="ExternalInput")
    h_out = nc.dram_tensor("o", [128, NBLK * DIM], mybir.dt.float32, kind="ExternalOutput")
    with tile.TileContext(nc) as tc:
        with tc.tile_pool(name="p", bufs=2) as pool:
            st = pool.tile([128, NBLK * DIM], mybir.dt.float32, tag="s")
            nc.sync.dma_start(out=st[:], in_=s_in.ap()[:])
            ct = pool.tile([128, NBLK * DIM], mybir.dt.float32, tag="c")
            nc.sync.dma_start(out=ct[:], in_=c_in.ap()[:])
            cm = pool.tile([128, NBLK * DIM], mybir.dt.float32, tag="cm")
            nc.vector.tensor_scalar_max(out=cm[:], in0=ct[:], scalar1=1.0)
            cr = pool.tile([128, NBLK * DIM], mybir.dt.float32, tag="cr")
            nc.vector.reciprocal(out=cr[:], in_=cm[:])
            ot = pool.tile([128, NBLK * DIM], mybir.dt.float32, tag="o")
            nc.vector.tensor_mul(out=ot[:], in0=st[:], in1=cr[:])
            nc.sync.dma_start(out=h_out.ap()[:], in_=ot[:])
    nc.compile()
    return nc


def _wrap_slots(arr):
    # [NS*128] slot-major -> [128, NS] partition-wrapped (edge e of slot s at
    # partition e, column s)
    ns = arr.shape[-1] // 128
    return np.ascontiguousarray(arr.reshape(ns, 128).T)


def _run_aggregate(prog, h_full, gsrc_w, glo_w):
    core_ids = list(range(N_CORES))
    iota_np = np.tile(np.arange(128, dtype=np.float32), (128, 1))
    in_maps = [{"h": h_full, "gsrc": gsrc_w[c], "glo": glo_w[c], "iotaf": iota_np}
               for c in range(N_CORES)]
    res = run_bass_kernel_spmd(prog, in_maps, core_ids)
    return [res.results[c]["sums"] for c in range(N_CORES)]


def _run_divide(prog, sums_list, cnts_list):
    core_ids = list(range(N_CORES))
    in_maps = [{"s": sums_list[c], "c": cnts_list[c]} for c in range(N_CORES)]
    res = run_bass_kernel_spmd(prog, in_maps, core_ids)
    h = np.empty((N_NODES, DIM), np.float32)
    for c in range(N_CORES):
        o = res.results[c]["o"].reshape(128, NBLK, DIM).transpose(1, 0, 2)
        h[c * NLOC:(c + 1) * NLOC] = o.reshape(NBLK * 128, DIM)[:NLOC]
    return h


def kernel(topic_entity_one_hot, edge_index, reverse_edge_index):
    x = np.asarray(topic_entity_one_hot, dtype=np.float32)
    shards = [_shard(np.asarray(edge_index)),
              _shard(np.asarray(reverse_edge_index))]
    U = max(s[2] for s in shards)
    # re-shard with the common U so both directions fit one program
    def repad(ei):
        gsrc, glo, _ = _shard_fixed(np.asarray(ei), U)
        return gsrc, glo
    fwd = repad(edge_index)
    rev = repad(reverse_edge_index)

    if ("agg", U) not in _PROGRAMS:
        _PROGRAMS[("agg", U)] = _build_aggregate(U)
    if "div" not in _PROGRAMS:
        _PROGRAMS["div"] = _build_divide()
    agg, div = _PROGRAMS[("agg", U)], _PROGRAMS["div"]

    results = []
    ones = np.ones((N_NODES, DIM), np.float32)
    for (gsrc, glo) in (fwd, rev):
        gsrc_w = [_wrap_slots(gsrc[c]) for c in range(N_CORES)]
        glo_w = [_wrap_slots(glo[c]) for c in range(N_CORES)]
        cnts = _run_aggregate(agg, ones, gsrc_w, glo_w)
        h = x
        for _ in range(2):
            sums = _run_aggregate(agg, h, gsrc_w, glo_w)
            h = _run_divide(div, sums, cnts)
            results.append(h)
    out = np.stack([results[0], results[1], results[2], results[3]], axis=0)
    return out


def _shard_fixed(edge_index, U):
    src = np.asarray(edge_index[0], dtype=np.int64)
    dst = np.asarray(edge_index[1], dtype=np.int64)
    core = dst // NLOC
    loc = dst - core * NLOC
    blk = loc // 128
    lo = loc % 128
    key = core * NBLK + blk
    order = np.argsort(key, kind='stable')
    ks, ss, ls = key[order], src[order], lo[order]
    counts = np.bincount(ks, minlength=N_CORES * NBLK).reshape(N_CORES, NBLK)
    assert counts.max() <= U * 128
    gsrc = np.zeros((N_CORES, NBLK * U * 128), np.int32)
    glo = np.full((N_CORES, NBLK * U * 128), PAD_LO, np.float32)
    starts = np.zeros(N_CORES * NBLK + 1, np.int64)
    np.cumsum(counts.ravel(), out=starts[1:])
    for c in range(N_CORES):
        for b in range(NBLK):
            k = c * NBLK + b
            n = counts[c, b]
            s0 = starts[k]
            base = (b * U) * 128
            gsrc[c, base:base + n] = ss[s0:s0 + n]
            glo[c, base:base + n] = ls[s0:s0 + n]
    return gsrc, glo, U



# revision 9
# speedup vs baseline: 1.9263x; 1.9263x over previous
"""GNN mean-aggregation (PyG MessagePassing, aggr='mean') on 8 TRN2 NeuronCores.

Reference computes 2 rounds of mean aggregation over edge_index plus 2 rounds
over reverse_edge_index on x [100000, 16]; output [4, 100000, 16].

Sharding (host): edges partitioned by destination across 8 cores (core c owns
dst in [12500c, 12500(c+1))). Within a core, edges are sorted by
(q = src mod 4, dst block of 128); each (q, block) run is padded to whole
128-edge chunks; each q region is padded to whole 1024-slot gather granules.
The chunk layout (U_qb) is shared across all 8 cores and both edge directions
so one compiled program serves every round.

Device program, per core per round:
  - dma_gather (SWDGE InstDMAGatherAnt, single_packet, 1024 idxs/instr):
    elements are 64 consecutive floats (256B) read from the full h table at
    byte offset q*64, index src//4 -- the first 16 floats of element
    (src//4, offset q*16 floats) are exactly h[src] when q == src%4. This
    turns the per-edge row gather into ~440 fast packetized gathers instead
    of per-row descriptors.
  - one-hot build over dst-lo via DVE is_equal against an iota tile
  - PE matmul accumulates one-hot^T @ msg into PSUM acc [128, 98*16]
  - fused divide: acc * recip (recip = 1/max(indegree,1) precomputed on host
    via bincount -- indegree depends only on edge indices)
Host reassembles the 8 per-core [12500, 16] slices into full h between rounds.
"""
import sys
sys.path.insert(0, '/opt/trn_rl_repo')
import numpy as np

import concourse.bass as bass
import concourse.tile as tile
from concourse import bacc, mybir
from concourse.bass_utils import run_bass_kernel_spmd
from concourse.library_config import mlp

N_NODES = 100000
DIM = 16
N_EDGES = 3200000
N_CORES = 8
NLOC = N_NODES // N_CORES        # 12500 nodes per core
NBLK = (NLOC + 127) // 128       # 98 dst blocks per core
NQ = 4                           # src mod 4 classes
GRAN = 1024                      # idxs per dma_gather (single_packet limit)
PAD_LO = 128.0                   # one-hot sentinel (never matches iota 0..127)
NGRP = N_NODES // 4              # 25000 4-node groups (int16-indexable)
HG_ROWS = NGRP + 1               # one pad group for the q-shift overhang

_PROGRAMS = {}


def _edge_fields(edge_index):
    src = np.asarray(edge_index[0], dtype=np.int64)
    dst = np.asarray(edge_index[1], dtype=np.int64)
    core = dst // NLOC
    loc = dst - core * NLOC
    return src, core, loc // 128, loc % 128, src % 4


def _counts_qb(edge_index):
    """[N_CORES, NQ, NBLK] edge counts."""
    src, core, blk, lo, q = _edge_fields(edge_index)
    key = (core * NQ + q) * NBLK + blk
    return np.bincount(key, minlength=N_CORES * NQ * NBLK).reshape(
        N_CORES, NQ, NBLK)


def _make_layout(count_list):
    """Shared chunk layout from per-direction [C, NQ, NBLK] counts."""
    cmax = np.maximum.reduce([c.max(axis=0) for c in count_list])  # [NQ, NBLK]
    U = (cmax + 127) // 128                                        # [NQ, NBLK]
    assert (U.sum(axis=0) >= 1).all()
    slots_q = U.sum(axis=1) * 128
    gran_q = -(-slots_q // GRAN)
    totq = gran_q * GRAN
    tot = int(totq.sum())
    ns = tot // 128
    # chunk schedule: sched[col] = (b, u) for real chunks
    sched = [None] * ns
    first_col = [None] * NBLK
    last_col = [None] * NBLK
    chunk_start_q = np.zeros(NQ + 1, np.int64)
    np.cumsum(totq // 128, out=chunk_start_q[1:])
    for q in range(NQ):
        col = int(chunk_start_q[q])
        for b in range(NBLK):
            for u in range(int(U[q, b])):
                sched[col] = (b, u)
                if first_col[b] is None:
                    first_col[b] = col
                last_col[b] = col
                col += 1
    return dict(U=U, slots_q=slots_q, gran_q=gran_q, totq=totq, tot=tot,
                ns=ns, sched=sched, first_col=first_col, last_col=last_col,
                chunk_start_q=chunk_start_q)


def _shard(edge_index, lay):
    """Per-core gidx [128, tot/16] int16, glo [128, ns] f32, recip [128,1568]."""
    src, core, blk, lo, q = _edge_fields(edge_index)
    key = (core * NQ + q) * NBLK + blk
    order = np.argsort(key, kind='stable')
    ks, gs, ls = key[order], (src[order] // 4).astype(np.int16), \
        lo[order].astype(np.float32)
    counts = np.bincount(key, minlength=N_CORES * NQ * NBLK)
    starts = np.zeros(N_CORES * NQ * NBLK + 1, np.int64)
    np.cumsum(counts, out=starts[1:])
    U = lay['U']
    tot, ns = lay['tot'], lay['ns']
    chunk_start_q = lay['chunk_start_q']
    gidx = np.zeros((N_CORES, tot), np.int16)
    glo = np.full((N_CORES, ns * 128), PAD_LO, np.float32)
    for c in range(N_CORES):
        for qq in range(NQ):
            col = int(chunk_start_q[qq])
            for b in range(NBLK):
                k = (c * NQ + qq) * NBLK + b
                n = counts[k]
                s0 = starts[k]
                base = col * 128
                gidx[c, base:base + n] = gs[s0:s0 + n]
                glo[c, base:base + n] = ls[s0:s0 + n]
                col += int(U[qq, b])
    # wrap gidx per 1024-granule: position i -> partition i%16, col i//16,
    # 16-partition block tiled 8x across the 128 partitions
    gidx_w = np.tile(
        gidx.reshape(N_CORES, tot // GRAN, GRAN // 16, 16)
            .transpose(0, 3, 1, 2)
            .reshape(N_CORES, 16, tot // 16),
        (1, 8, 1))
    # glo wrap: chunk col-major [128, ns]
    glo_w = np.ascontiguousarray(
        glo.reshape(N_CORES, ns, 128).transpose(0, 2, 1))
    # recip of in-degree per core
    dst = np.asarray(edge_index[1], dtype=np.int64)
    cnt = np.bincount(dst, minlength=N_NODES).astype(np.float32)
    recip = 1.0 / np.maximum(cnt, 1.0)
    rec = np.ones((N_CORES, NBLK * 128), np.float32)
    rec[:, :NLOC] = recip.reshape(N_CORES, NLOC)
    rec_w = np.repeat(rec.reshape(N_CORES, NBLK, 128).transpose(0, 2, 1),
                      DIM, axis=2)      # [C, 128, NBLK*16]
    return (np.ascontiguousarray(gidx_w),
            glo_w,
            np.ascontiguousarray(rec_w.astype(np.float32)))


def _build_program(lay, repeat=1):
    tot, ns = lay['tot'], lay['ns']
    gran_q, chunk_start_q = lay['gran_q'], lay['chunk_start_q']
    sched, first_col, last_col = lay['sched'], lay['first_col'], lay['last_col']
    nc = bacc.Bacc("TRN2", target_bir_lowering=False, debug=False,
                   num_devices=N_CORES)
    hg = nc.dram_tensor("hg", [HG_ROWS, 64], mybir.dt.float32,
                        kind="ExternalInput")
    gidx_t = nc.dram_tensor("gidx", [128, tot // 16], mybir.dt.int16,
                            kind="ExternalInput")
    glo_t = nc.dram_tensor("glo", [128, ns], mybir.dt.float32,
                           kind="ExternalInput")
    iotaf = nc.dram_tensor("iotaf", [128, 128], mybir.dt.float32,
                           kind="ExternalInput")
    recip_t = nc.dram_tensor("recip", [128, NBLK * DIM], mybir.dt.float32,
                             kind="ExternalInput")
    out_t = nc.dram_tensor("out", [128, NBLK * DIM], mybir.dt.float32,
                           kind="ExternalOutput")
    with tile.TileContext(nc) as tc:
        with (
            tc.tile_pool(name="const", bufs=1) as constp,
            tc.tile_pool(name="gat", bufs=12) as gpool,
            tc.tile_pool(name="oh", bufs=8) as ohp,
            tc.tile_pool(name="fin", bufs=1) as finp,
            tc.tile_pool(name="psum", bufs=1, space="PSUM") as psump,
        ):
            nc.gpsimd.load_library(mlp)
            iota = constp.tile([128, 128], mybir.dt.float32)
            nc.sync.dma_start(out=iota[:], in_=iotaf.ap()[:])
            glo_s = constp.tile([128, ns], mybir.dt.float32)
            nc.sync.dma_start(out=glo_s[:], in_=glo_t.ap()[:])
            recip_s = constp.tile([128, NBLK * DIM], mybir.dt.float32)
            nc.sync.dma_start(out=recip_s[:], in_=recip_t.ap()[:])
            gidx_s = constp.tile([128, tot // 16], mybir.dt.int16)
            nc.sync.dma_start(out=gidx_s[:], in_=gidx_t.ap()[:])
            acc = psump.tile([128, NBLK * DIM], mybir.dt.float32, space="PSUM")
            flat = hg.ap().flatten()
            U = lay['U']
            in_aps = [flat[q * 16: q * 16 + NGRP * 64].rearrange(
                "(a b) -> a b", b=64) for q in range(NQ)]
            for _rep in range(repeat):
                # b-major chunk order: each block's PSUM accumulation group is
                # contiguous (HW allows only one open group per bank). Gathers
                # (q-stream granules) are issued lazily as their chunks come up.
                gtiles = {}

                def ensure_gather(q, g):
                    if (q, g) in gtiles:
                        return gtiles[(q, g)]
                    gt = gpool.tile([128, 8 * 64], mybir.dt.float32, tag="gt")
                    icol = (int(chunk_start_q[q]) * 128 + g * GRAN) // 16
                    nc.gpsimd.dma_gather(
                        gt[:].rearrange("p (c e) -> p c e", e=64),
                        in_aps[q], gidx_s[:, icol:icol + GRAN // 16],
                        GRAN, GRAN, 64, single_packet=True)
                    gtiles[(q, g)] = gt
                    return gt

                # chunk col for (q, b, u)
                ucum = np.zeros((NQ, NBLK + 1), np.int64)
                for q in range(NQ):
                    np.cumsum(U[q], out=ucum[q, 1:])
                nu = U.sum(axis=0)  # chunks per block
                for b in range(NBLK):
                    done = 0
                    for q in range(NQ):
                        cbase = int(chunk_start_q[q])
                        for u in range(int(U[q, b])):
                            col = cbase + int(ucum[q, b]) + u
                            g = (col - cbase) // 8
                            k = (col - cbase) % 8
                            gt = ensure_gather(q, g)
                            oh = ohp.tile([128, 128], mybir.dt.float32,
                                          tag="oh")
                            nc.vector.tensor_tensor(
                                out=oh[:],
                                in0=glo_s[:, col:col + 1].to_broadcast(
                                    [128, 128]),
                                in1=iota[:], op=mybir.AluOpType.is_equal)
                            nc.tensor.matmul(
                                out=acc[:, b * DIM:(b + 1) * DIM],
                                lhsT=oh[:], rhs=gt[:, k * 64:k * 64 + DIM],
                                start=(done == 0),
                                stop=(done == int(nu[b]) - 1))
                            done += 1
            accs = finp.tile([128, NBLK * DIM], mybir.dt.float32)
            nc.vector.tensor_copy(out=accs[:], in_=acc[:])
            outv = finp.tile([128, NBLK * DIM], mybir.dt.float32)
            nc.vector.tensor_mul(out=outv[:], in0=accs[:], in1=recip_s[:])
            nc.sync.dma_start(out=out_t.ap()[:], in_=outv[:])
    nc.compile()
    return nc


def _build_program_repeat(lay, repeat):
    return _build_program(lay, repeat=repeat)


def _pad_h(h):
    hp = np.zeros((HG_ROWS * 4, DIM), np.float32)
    hp[:N_NODES] = h
    return hp.reshape(HG_ROWS, 64)


_IOTA = np.tile(np.arange(128, dtype=np.float32), (128, 1))


def _run_round(prog, h_full, shards):
    gidx_w, glo_w, rec_w = shards
    hg = _pad_h(h_full)
    in_maps = [{"hg": hg, "gidx": gidx_w[c], "glo": glo_w[c],
                "iotaf": _IOTA, "recip": rec_w[c]} for c in range(N_CORES)]
    res = run_bass_kernel_spmd(prog, in_maps, list(range(N_CORES)))
    h = np.empty((N_NODES, DIM), np.float32)
    for c in range(N_CORES):
        o = res.results[c]["out"].reshape(128, NBLK, DIM).transpose(1, 0, 2)
        h[c * NLOC:(c + 1) * NLOC] = o.reshape(NBLK * 128, DIM)[:NLOC]
    return h


def kernel(topic_entity_one_hot, edge_index, reverse_edge_index):
    x = np.asarray(topic_entity_one_hot, dtype=np.float32)
    ef = np.asarray(edge_index)
    er = np.asarray(reverse_edge_index)
    lay = _make_layout([_counts_qb(ef), _counts_qb(er)])
    key = ("agg", lay['tot'], tuple(lay['U'].ravel().tolist()))
    if key not in _PROGRAMS:
        _PROGRAMS[key] = _build_program(lay)
    prog = _PROGRAMS[key]
    _PROGRAMS['_last_layout'] = lay

    results = []
    for ei in (ef, er):
        shards = _shard(ei, lay)
        h = x
        for _ in range(2):
            h = _run_round(prog, h, shards)
            results.append(h)
    return np.stack(results, axis=0)
